# revision 10
# baseline (speedup 1.0000x reference)
"""MinGRU cell kernel for Trainium2 (8 NeuronCores, data-parallel over batch).

Computes, for x:[B,T,D], motion_mag:[B,T]:
    tau = 1 + softplus(alpha) * sigmoid(mw*mm + mb)        (per b,t)
    z   = sigmoid((x @ Wz^T + bz) / tau)                   (B,T,H)
    ht  = x @ Wh^T + bh                                    (B,T,H)
    h_t = (1-z_t)*h_{t-1} + z_t*ht_t   (scan over t, h_0=0)

Strategy:
  - Shard B=32 across 8 cores (4 per core). Weights replicated (bf16).
  - On-chip layout: h on partitions, t on the free dim, so the recurrence is
    a HW tensor_tensor_scan per [128h, 1024t] tile, carried across t-tiles via
    initial=prev[:, -1:].
  - The gate temperature is folded into the z-GEMM so no device op ever
    touches invtau: the host uploads x_z = x*invtau (bf16) for the z
    projection and x_h = x (bf16) for the candidate projection, and the
    column-dependent bias bz*invtau rides as a 513th contraction row
    (K=1 matmul: lhsT = bz row, rhs = invtau row). PSUM_z then directly
    holds sigmoid's argument.
  - Post-GEMM dataflow is one-directional with minimal cross-engine
    coupling (the DVE scan is fixed at 2 cyc/elem, so the DVE gets only
    work that must be there):
      ACT     : z = sigmoid(zq)    (PSUM->SBUF bf16; evacuates z psum)
      ACT     : a = 1 - z          (Identity, scale=-1, bias=1)
      DVE STT : b = (hq + bh) * z  (PSUM read; evacuates h psum)
      DVE scan: h = scan(a, b)     (bf16 io, fp32 state)
    Tensor (18 matmuls/tile) is the pacing engine; DVE/ACT run with slack.
  - The output DMA is bf16; the host casts back to fp32.
  - A few dummy fp32 matmuls at t=0 warm the PE HAM clock-gate (2.4 GHz)
    while the first weight/x DMAs land.
  - Host pre-transposes x to [d, b*t] per core and un-transposes the output.
"""

import sys

import numpy as np

if "/opt/trn_rl_repo" not in sys.path:
    sys.path.insert(0, "/opt/trn_rl_repo")

B, T, D, H = 32, 2048, 512, 512
NCORES = 8
BL = B // NCORES            # batch per core = 4
TBLK = 1024                 # t-columns per tile
MMN = 512                   # matmul free-dim (1 psum bank)
NTB = T // TBLK             # 2 t-blocks per sample
DC = D // 128               # 4 contraction chunks
HC = H // 128               # 4 h partition chunks
BT = BL * T                 # 8192 columns per core

_CACHE = {}


def _build_nc(bh0=None):
    import concourse.bass as bass
    import concourse.bacc as bacc
    import concourse.mybir as mybir
    import concourse.tile as tile
    from contextlib import ExitStack

    f32 = mybir.dt.float32
    bf16 = mybir.dt.bfloat16
    AF = mybir.ActivationFunctionType
    OP = mybir.AluOpType

    nc = bacc.Bacc("TRN2", target_bir_lowering=False, debug=False)

    xz_ext = nc.declare_dram_parameter("xz", [DC, 128, BT], bf16, isOutput=False)
    xh_ext = nc.declare_dram_parameter("xh", [DC, 128, BT], bf16, isOutput=False)
    wzt_ext = nc.declare_dram_parameter("wzt", [HC, 128, DC, 128], bf16, isOutput=False)
    wht_ext = nc.declare_dram_parameter("wht", [HC, 128, DC, 128], bf16, isOutput=False)
    bzr_ext = nc.declare_dram_parameter("bzr", [HC, 1, 128], bf16, isOutput=False)
    bh_ext = nc.declare_dram_parameter("bh", [HC, 128, 1], f32, isOutput=False)
    itr_ext = nc.declare_dram_parameter("invtau", [1, BT], bf16, isOutput=False)
    out_ext = nc.declare_dram_parameter("out", [BL, HC, 128, T], bf16, isOutput=True)

    with tile.TileContext(nc) as tc, ExitStack() as ctx:
        singles = ctx.enter_context(tc.tile_pool(name="singles", bufs=1))
        x_pool = ctx.enter_context(tc.tile_pool(name="x", bufs=3))
        psum = ctx.enter_context(tc.tile_pool(name="psum", bufs=2, space="PSUM"))
        work = ctx.enter_context(tc.tile_pool(name="work", bufs=4))
        ab_pool = ctx.enter_context(tc.tile_pool(name="ab", bufs=4))
        h_pool = ctx.enter_context(tc.tile_pool(name="h", bufs=8))

        # HAM warm-up: dependency-free fp32 matmuls (1 col / 4 cycles, so
        # each is long) keep the PE busy while the first weight/x DMAs land,
        # flipping the clock-gate to 8/8 before the real GEMMs start.
        warm = singles.tile([128, MMN], f32, tag="warm", name="warm")
        nc.vector.memset(warm[:], 0.0)
        # Dummy activation: triggers the ~2.7us ACT table load during the
        # initial DMA window instead of on the first tile's critical path.
        warmact = singles.tile([128, 1], bf16, tag="warmact", name="warmact")
        nc.scalar.activation(warmact[:], warm[:, 0:1], AF.Sigmoid)
        wq0 = psum.tile([128, MMN], f32, tag="zq", name="warmq")
        for i in range(3):
            nc.tensor.matmul(
                wq0[:], lhsT=warm[:, 0:128], rhs=warm[:], start=True, stop=True
            )

        # Weights are hc-major in DRAM: the first matmul group (hc=0) only
        # needs a 128KB DMA. First block's x arrives as 512-col halves so the
        # first 4-matmul group is gated on ~0.6MB instead of 2MB.
        wz_hc, wh_hc = [None] * HC, [None] * HC
        xz0h = [[None] * DC for _ in range(2)]
        xh0h = [[None] * DC for _ in range(2)]
        # invtau row (partition 0 only) + per-hc bz rows for the bias matmul.
        itrow = singles.tile([1, BT], bf16, tag="itrow", name="itrow")
        nc.sync.dma_start(out=itrow[:, 0:TBLK], in_=itr_ext[:, 0:TBLK])
        bzr = []
        for hc in range(HC):
            r = singles.tile([1, 128], bf16, tag=f"bzr{hc}", name=f"bzr{hc}")
            nc.sync.dma_start(out=r[:], in_=bzr_ext[hc])
            bzr.append(r)
        wz_hc[0] = singles.tile([128, DC * 128], bf16, tag="wzhc0", name="wzhc0")
        nc.sync.dma_start(out=wz_hc[0][:], in_=wzt_ext[0])
        for dc in range(DC):
            xt = x_pool.tile([128, MMN], bf16, tag=f"xz{dc}", name=f"xz0a_{dc}")
            nc.sync.dma_start(out=xt[:], in_=xz_ext[dc, :, 0:MMN])
            xz0h[0][dc] = xt
        wh_hc[0] = singles.tile([128, DC * 128], bf16, tag="whhc0", name="whhc0")
        nc.sync.dma_start(out=wh_hc[0][:], in_=wht_ext[0])
        for dc in range(DC):
            xt = x_pool.tile([128, MMN], bf16, tag=f"xh{dc}", name=f"xh0a_{dc}")
            nc.sync.dma_start(out=xt[:], in_=xh_ext[dc, :, 0:MMN])
            xh0h[0][dc] = xt
        for dc in range(DC):
            xt = x_pool.tile([128, MMN], bf16, tag=f"xz{dc}", name=f"xz0b_{dc}")
            nc.sync.dma_start(out=xt[:], in_=xz_ext[dc, :, MMN:TBLK])
            xz0h[1][dc] = xt
            xt = x_pool.tile([128, MMN], bf16, tag=f"xh{dc}", name=f"xh0b_{dc}")
            nc.sync.dma_start(out=xt[:], in_=xh_ext[dc, :, MMN:TBLK])
            xh0h[1][dc] = xt
        for hc in range(1, HC):
            w = singles.tile([128, DC * 128], bf16, tag=f"wzhc{hc}", name=f"wzhc{hc}")
            nc.sync.dma_start(out=w[:], in_=wzt_ext[hc])
            wz_hc[hc] = w
            w = singles.tile([128, DC * 128], bf16, tag=f"whhc{hc}", name=f"whhc{hc}")
            nc.sync.dma_start(out=w[:], in_=wht_ext[hc])
            wh_hc[hc] = w
        # rest of the invtau row (tiny; after the hot first-block DMAs)
        nc.sync.dma_start(out=itrow[:, TBLK:BT], in_=itr_ext[:, TBLK:BT])

        # b-STT bias: uniform bh rides as an immediate, else [128,1] columns.
        if bh0 is not None:
            bh_col = [bh0] * HC
        else:
            bh_col = []
            for hc in range(HC):
                bc = singles.tile([128, 1], f32, tag=f"bh{hc}", name=f"bh{hc}")
                nc.gpsimd.dma_start(out=bc[:], in_=bh_ext[hc])
                bh_col.append(bc[:])

        # Tile schedule: 32 tiles of [128h, 1024t], hc-inner.
        sched = []
        for b in range(BL):
            for tb in range(NTB):
                for hc in range(HC):
                    first_blk = (b == 0 and tb == 0)
                    last = (b == BL - 1 and tb == NTB - 1 and hc == HC - 1)
                    nsub = 2 if ((first_blk and hc == 0) or last) else 1
                    sched.append((b, tb, hc, nsub))
        NT = len(sched)

        xs_blocks = {}
        state = [None] * NT
        h_prev = [[None] * HC for _ in range(BL)]

        def emit_mms(i):
            """x DMAs (at block starts) + z-GEMM (with bias row) + h-GEMM."""
            b, tb, hc, nsub = sched[i]
            k = b * NTB + tb
            first_blk = (k == 0)
            bt0 = b * T + tb * TBLK
            if hc == 0 and not first_blk:
                xzs, xhs = [], []
                for dc in range(DC):
                    xt = x_pool.tile([128, TBLK], bf16, tag=f"xz{dc}")
                    nc.sync.dma_start(out=xt[:], in_=xz_ext[dc, :, bt0:bt0 + TBLK])
                    xzs.append(xt)
                for dc in range(DC):
                    xt = x_pool.tile([128, TBLK], bf16, tag=f"xh{dc}")
                    nc.sync.dma_start(out=xt[:], in_=xh_ext[dc, :, bt0:bt0 + TBLK])
                    xhs.append(xt)
                xs_blocks[k] = (xzs, xhs)
            xz, xh = (None, None) if first_blk else xs_blocks[k]
            zq = psum.tile([128, TBLK], f32, tag="zq")
            for half in range(2):
                psl = slice(half * MMN, (half + 1) * MMN)
                for dc in range(DC):
                    rhs = (xz0h[half][dc][:] if first_blk else xz[dc][:, psl])
                    nc.tensor.matmul(
                        zq[:, psl],
                        lhsT=wz_hc[hc][:, dc * 128:(dc + 1) * 128],
                        rhs=rhs,
                        start=(dc == 0),
                        stop=False,
                    )
                # bias row: zq += bz ⊗ invtau  (K=1 matmul)
                nc.tensor.matmul(
                    zq[:, psl],
                    lhsT=bzr[hc][:],
                    rhs=itrow[:, bt0 + half * MMN:bt0 + (half + 1) * MMN],
                    start=False,
                    stop=True,
                )
            hq = psum.tile([128, TBLK], f32, tag="hq")
            for half in range(2):
                psl = slice(half * MMN, (half + 1) * MMN)
                for dc in range(DC):
                    rhs = (xh0h[half][dc][:] if first_blk else xh[dc][:, psl])
                    nc.tensor.matmul(
                        hq[:, psl],
                        lhsT=wh_hc[hc][:, dc * 128:(dc + 1) * 128],
                        rhs=rhs,
                        start=(dc == 0),
                        stop=(dc == DC - 1),
                    )
            state[i] = {"zq": zq, "hq": hq}

        def emit_zs(i):
            """ACT: z = sigmoid(zq) (evacuates z psum), a = 1 - z."""
            b, tb, hc, nsub = sched[i]
            st = state[i]
            z = work.tile([128, TBLK], bf16, tag="z")
            a = ab_pool.tile([128, TBLK], bf16, tag="a")
            width = TBLK // nsub
            for sub in range(nsub):
                ssl = slice(sub * width, (sub + 1) * width)
                nc.scalar.activation(z[:, ssl], st["zq"][:, ssl], AF.Sigmoid)
                nc.scalar.activation(
                    a[:, ssl], z[:, ssl], AF.Identity, bias=1.0, scale=-1.0
                )
            st["z"], st["a"] = z, a

        def emit_back(i):
            """DVE: b = (hq + bh) * z (evacuates h psum), scan, out-DMA."""
            b, tb, hc, nsub = sched[i]
            st = state[i]
            z, a = st["z"], st["a"]
            bb = ab_pool.tile([128, TBLK], bf16, tag="b")
            h = h_pool.tile([128, TBLK], bf16, tag="h")
            width = TBLK // nsub
            for sub in range(nsub):
                ssl = slice(sub * width, (sub + 1) * width)
                nc.vector.scalar_tensor_tensor(
                    bb[:, ssl], st["hq"][:, ssl], bh_col[hc], z[:, ssl],
                    op0=OP.add, op1=OP.mult,
                )
                init = (
                    (0.0 if tb == 0 else h_prev[b][hc][:, TBLK - 1:TBLK])
                    if sub == 0 else h[:, sub * width - 1:sub * width]
                )
                nc.vector.tensor_tensor_scan(
                    h[:, ssl], a[:, ssl], bb[:, ssl], init,
                    op0=OP.mult, op1=OP.add,
                )
                if nsub > 1:
                    osl = slice(tb * TBLK + sub * width,
                                tb * TBLK + (sub + 1) * width)
                    nc.sync.dma_start(out=out_ext[b, hc, :, osl], in_=h[:, ssl])
            h_prev[b][hc] = h
            if nsub == 1:
                ts = slice(tb * TBLK, (tb + 1) * TBLK)
                nc.sync.dma_start(out=out_ext[b, hc, :, ts], in_=h[:])
            state[i] = None

        # Tensor runs one tile ahead; ACT (sigma/a) and DVE (b/scan) trail.
        emit_mms(0)
        for i in range(NT):
            if i + 1 < NT:
                emit_mms(i + 1)
            emit_zs(i)
            emit_back(i)

    nc.compile()
    return nc


def _prep_inputs(x, motion_mag, Wz, bz, Wh, bh, motion_weight, motion_bias, alpha):
    import ml_dtypes

    bf = ml_dtypes.bfloat16
    x = np.ascontiguousarray(np.asarray(x, dtype=np.float32))
    mm = np.asarray(motion_mag, dtype=np.float32)
    Wz = np.asarray(Wz, dtype=np.float32)
    Wh = np.asarray(Wh, dtype=np.float32)
    bz = np.asarray(bz, dtype=np.float32)
    bh = np.asarray(bh, dtype=np.float32).reshape(HC, 128, 1)
    mw = float(np.asarray(motion_weight))
    mb = float(np.asarray(motion_bias))
    al = float(np.asarray(alpha))

    a_sp = float(np.log1p(np.exp(al)))  # softplus(alpha)
    sig = 1.0 / (1.0 + np.exp(-(mw * mm + mb)))
    invtau = (1.0 / (1.0 + a_sp * sig)).astype(np.float32)  # [B, T]

    wzt = np.ascontiguousarray(
        Wz.T.reshape(DC, 128, HC, 128).transpose(2, 1, 0, 3)).astype(bf)
    wht = np.ascontiguousarray(
        Wh.T.reshape(DC, 128, HC, 128).transpose(2, 1, 0, 3)).astype(bf)
    bzr = np.ascontiguousarray(bz.reshape(HC, 1, 128)).astype(bf)

    in_maps = []
    for c in range(NCORES):
        xl = x[c * BL:(c + 1) * BL].reshape(BL * T, D)
        xt = np.ascontiguousarray(xl.T)                      # [D, BT] f32
        itc = np.ascontiguousarray(
            invtau[c * BL:(c + 1) * BL]).reshape(1, BT)      # [1, BT]
        xzt = (xt * itc).astype(bf).reshape(DC, 128, BT)
        xht = xt.astype(bf).reshape(DC, 128, BT)
        in_maps.append({
            "xz": xzt,
            "xh": xht,
            "wzt": wzt,
            "wht": wht,
            "bzr": bzr,
            "bh": bh,
            "invtau": itc.astype(bf),
        })
    return in_maps


def _assemble(results):
    outs = []
    for c in range(NCORES):
        o = results[c]["out"]  # [BL, HC, 128, T] bf16
        o = np.transpose(o.astype(np.float32), (0, 3, 1, 2)).reshape(BL, T, H)
        outs.append(o)
    return np.ascontiguousarray(np.concatenate(outs, axis=0))


def _run(inputs, trace=False):
    from concourse.bass_utils import run_bass_kernel_spmd

    bha = np.asarray(inputs["bh"], dtype=np.float32).reshape(-1)
    bh0 = float(bha[0]) if np.all(bha == bha[0]) else None
    key = ("nc", bh0)
    if key not in _CACHE:
        _CACHE[key] = _build_nc(bh0)
    nc = _CACHE[key]
    in_maps = _prep_inputs(**inputs)
    res = run_bass_kernel_spmd(nc, in_maps, list(range(NCORES)), trace=trace)
    return _assemble(res.results), res


def kernel(**inputs):
    out, _ = _run(inputs, trace=False)
    return out


# revision 14
# speedup vs baseline: 1.0545x; 1.0545x over previous
"""MinGRU cell kernel for Trainium2 (8 NeuronCores, data-parallel over batch).

Computes, for x:[B,T,D], motion_mag:[B,T]:
    tau = 1 + softplus(alpha) * sigmoid(mw*mm + mb)        (per b,t)
    z   = sigmoid((x @ Wz^T + bz) / tau)                   (B,T,H)
    ht  = x @ Wh^T + bh                                    (B,T,H)
    h_t = (1-z_t)*h_{t-1} + z_t*ht_t   (scan over t, h_0=0)

Strategy:
  - Shard B=32 across 8 cores (4 per core). Weights replicated (bf16).
  - On-chip layout: h on partitions, t on the free dim, so the recurrence is
    a HW tensor_tensor_scan per [128h, 1024t] tile, carried across t-tiles via
    initial=prev[:, -1:].
  - The gate temperature is folded into the z-GEMM so no device op ever
    touches invtau: the host uploads x_z = x*invtau (bf16) for the z
    projection and x_h = x (bf16) for the candidate projection, and the
    column-dependent bias bz*invtau rides as a 513th contraction row
    (K=1 matmul: lhsT = bz row, rhs = invtau row). PSUM_z then directly
    holds sigmoid's argument.
  - Post-GEMM dataflow is one-directional with minimal cross-engine
    coupling (the DVE scan is fixed at 2 cyc/elem, so the DVE gets only
    work that must be there):
      ACT     : z = sigmoid(zq)    (PSUM->SBUF bf16; evacuates z psum)
      ACT     : a = 1 - z          (Identity, scale=-1, bias=1)
      DVE STT : b = (hq + bh) * z  (PSUM read; evacuates h psum)
      DVE scan: h = scan(a, b)     (bf16 io, fp32 state)
    Tensor (18 matmuls/tile) is the pacing engine; DVE/ACT run with slack.
  - The output DMA is bf16; the host casts back to fp32.
  - A few dummy fp32 matmuls at t=0 warm the PE HAM clock-gate (2.4 GHz)
    while the first weight/x DMAs land.
  - Host pre-transposes x to [d, b*t] per core and un-transposes the output.
"""

import sys

import numpy as np

if "/opt/trn_rl_repo" not in sys.path:
    sys.path.insert(0, "/opt/trn_rl_repo")

B, T, D, H = 32, 2048, 512, 512
NCORES = 8
BL = B // NCORES            # batch per core = 4
TBLK = 1024                 # t-columns per tile
MMN = 512                   # matmul free-dim (1 psum bank)
NTB = T // TBLK             # 2 t-blocks per sample
DC = D // 128               # 4 contraction chunks
HC = H // 128               # 4 h partition chunks
BT = BL * T                 # 8192 columns per core

_CACHE = {}


def _build_nc(bh0=None):
    import concourse.bass as bass
    import concourse.bacc as bacc
    import concourse.mybir as mybir
    import concourse.tile as tile
    from contextlib import ExitStack

    f32 = mybir.dt.float32
    bf16 = mybir.dt.bfloat16
    AF = mybir.ActivationFunctionType
    OP = mybir.AluOpType

    nc = bacc.Bacc("TRN2", target_bir_lowering=False, debug=False)

    xz_ext = nc.declare_dram_parameter("xz", [BL * NTB, DC, 128, TBLK], bf16, isOutput=False)
    xh_ext = nc.declare_dram_parameter("xh", [BL * NTB, DC, 128, TBLK], bf16, isOutput=False)
    wzt_ext = nc.declare_dram_parameter("wzt", [HC, 128, DC, 128], bf16, isOutput=False)
    wht_ext = nc.declare_dram_parameter("wht", [HC, 128, DC, 128], bf16, isOutput=False)
    bzr_ext = nc.declare_dram_parameter("bzr", [HC, 1, 128], bf16, isOutput=False)
    bh_ext = nc.declare_dram_parameter("bh", [HC, 128, 1], f32, isOutput=False)
    itr_ext = nc.declare_dram_parameter("invtau", [1, BT], bf16, isOutput=False)
    out_ext = nc.declare_dram_parameter("out", [BL, HC, 128, T], bf16, isOutput=True)

    with tile.TileContext(nc) as tc, ExitStack() as ctx:
        singles = ctx.enter_context(tc.tile_pool(name="singles", bufs=1))
        x_pool = ctx.enter_context(tc.tile_pool(name="x", bufs=3))
        psum = ctx.enter_context(tc.tile_pool(name="psum", bufs=2, space="PSUM"))
        work = ctx.enter_context(tc.tile_pool(name="work", bufs=4))
        ab_pool = ctx.enter_context(tc.tile_pool(name="ab", bufs=4))
        h_pool = ctx.enter_context(tc.tile_pool(name="h", bufs=8))

        # HAM warm-up: dependency-free fp32 matmuls (1 col / 4 cycles, so
        # each is long) keep the PE busy while the first weight/x DMAs land,
        # flipping the clock-gate to 8/8 before the real GEMMs start.
        warm = singles.tile([128, MMN], f32, tag="warm", name="warm")
        nc.vector.memset(warm[:], 0.0)
        # Dummy activation: triggers the ~2.7us ACT table load during the
        # initial DMA window instead of on the first tile's critical path.
        warmact = singles.tile([128, 1], bf16, tag="warmact", name="warmact")
        nc.scalar.activation(warmact[:], warm[:, 0:1], AF.Sigmoid)
        wq0 = psum.tile([128, MMN], f32, tag="zq", name="warmq")
        for i in range(3):
            nc.tensor.matmul(
                wq0[:], lhsT=warm[:, 0:128], rhs=warm[:], start=True, stop=True
            )

        # Weights are hc-major in DRAM: the first matmul group (hc=0) only
        # needs a 128KB DMA. First block's x arrives as 512-col halves so the
        # first 4-matmul group is gated on ~0.6MB instead of 2MB.
        wz_hc, wh_hc = [None] * HC, [None] * HC
        # invtau row (partition 0 only) + per-hc bz rows for the bias matmul.
        itrow = singles.tile([1, BT], bf16, tag="itrow", name="itrow")
        nc.sync.dma_start(out=itrow[:, 0:TBLK], in_=itr_ext[:, 0:TBLK])
        bzr = []
        for hc in range(HC):
            r = singles.tile([1, 128], bf16, tag=f"bzr{hc}", name=f"bzr{hc}")
            nc.sync.dma_start(out=r[:], in_=bzr_ext[hc])
            bzr.append(r)
        def x_half_dma(tile_, ext, k, half):
            # cols [half*MMN, (half+1)*MMN) of every dc chunk of block k, in
            # one strided DMA: dst tile[:, dc*TBLK + half*MMN + c].
            in_ = ext[k, :, :, half * MMN:(half + 1) * MMN]
            in_p = bass.AP(
                tensor=in_.tensor, offset=in_.offset,
                ap=[list(in_.ap[1]), list(in_.ap[0]), list(in_.ap[2])],
            )
            t_ap = tile_[:, half * MMN::TBLK]   # [128, DC] strided view
            out_p = bass.AP(
                tensor=t_ap.tensor, offset=t_ap.offset,
                ap=list(t_ap.ap) + [[1, MMN]],
            )
            nc.sync.dma_start(out=out_p, in_=in_p)

        wz_hc[0] = singles.tile([128, DC * 128], bf16, tag="wzhc0", name="wzhc0")
        nc.sync.dma_start(out=wz_hc[0][:], in_=wzt_ext[0])
        xz0 = x_pool.tile([128, DC * TBLK], bf16, tag="xz", name="xz0")
        x_half_dma(xz0, xz_ext, 0, 0)
        wh_hc[0] = singles.tile([128, DC * 128], bf16, tag="whhc0", name="whhc0")
        nc.sync.dma_start(out=wh_hc[0][:], in_=wht_ext[0])
        xh0 = x_pool.tile([128, DC * TBLK], bf16, tag="xh", name="xh0")
        x_half_dma(xh0, xh_ext, 0, 0)
        x_half_dma(xz0, xz_ext, 0, 1)
        x_half_dma(xh0, xh_ext, 0, 1)
        for hc in range(1, HC):
            w = singles.tile([128, DC * 128], bf16, tag=f"wzhc{hc}", name=f"wzhc{hc}")
            nc.sync.dma_start(out=w[:], in_=wzt_ext[hc])
            wz_hc[hc] = w
            w = singles.tile([128, DC * 128], bf16, tag=f"whhc{hc}", name=f"whhc{hc}")
            nc.sync.dma_start(out=w[:], in_=wht_ext[hc])
            wh_hc[hc] = w
        # rest of the invtau row (tiny; after the hot first-block DMAs)
        nc.sync.dma_start(out=itrow[:, TBLK:BT], in_=itr_ext[:, TBLK:BT])

        # b-STT bias: uniform bh rides as an immediate, else [128,1] columns.
        if bh0 is not None:
            bh_col = [bh0] * HC
        else:
            bh_col = []
            for hc in range(HC):
                bc = singles.tile([128, 1], f32, tag=f"bh{hc}", name=f"bh{hc}")
                nc.gpsimd.dma_start(out=bc[:], in_=bh_ext[hc])
                bh_col.append(bc[:])

        # Tile schedule: 32 tiles of [128h, 1024t], hc-inner.
        sched = []
        for b in range(BL):
            for tb in range(NTB):
                for hc in range(HC):
                    first_blk = (b == 0 and tb == 0)
                    last = (b == BL - 1 and tb == NTB - 1 and hc == HC - 1)
                    nsub = 2 if ((first_blk and hc == 0) or last) else 1
                    sched.append((b, tb, hc, nsub))
        NT = len(sched)

        xs_blocks = {}
        state = [None] * NT
        h_prev = [[None] * HC for _ in range(BL)]

        xs_blocks[0] = (xz0, xh0)

        def emit_mms(i):
            """x prefetch (one block ahead) + z-GEMM (with bias row) + h-GEMM."""
            b, tb, hc, nsub = sched[i]
            k = b * NTB + tb
            bt0 = b * T + tb * TBLK
            if hc == 0 and k + 1 < BL * NTB:
                kn = k + 1

                def x_blk_dma(ext):
                    # [DC,128,TBLK] dc-major in DRAM -> [128, DC*TBLK] tile,
                    # partition dim first on both sides of the transfer.
                    xt = x_pool.tile([128, DC * TBLK], bf16,
                                     tag="xz" if ext is xz_ext else "xh")
                    in_ = ext[kn]
                    in_p = bass.AP(
                        tensor=in_.tensor, offset=in_.offset,
                        ap=[list(in_.ap[1]), list(in_.ap[0]), list(in_.ap[2])],
                    )
                    t_ap = xt[:]
                    out_p = bass.AP(
                        tensor=t_ap.tensor, offset=t_ap.offset,
                        ap=[list(t_ap.ap[0]), [TBLK, DC], [1, TBLK]],
                    )
                    nc.sync.dma_start(out=out_p, in_=in_p)
                    return xt

                xs_blocks[kn] = (x_blk_dma(xz_ext), x_blk_dma(xh_ext))
            xz, xh = xs_blocks[k]
            zq = psum.tile([128, TBLK], f32, tag="zq")
            for half in range(2):
                psl = slice(half * MMN, (half + 1) * MMN)
                for dc in range(DC):
                    csl = slice(dc * TBLK + half * MMN, dc * TBLK + (half + 1) * MMN)
                    nc.tensor.matmul(
                        zq[:, psl],
                        lhsT=wz_hc[hc][:, dc * 128:(dc + 1) * 128],
                        rhs=xz[:, csl],
                        start=(dc == 0),
                        stop=False,
                    )
                # bias row: zq += bz ⊗ invtau  (K=1 matmul)
                nc.tensor.matmul(
                    zq[:, psl],
                    lhsT=bzr[hc][:],
                    rhs=itrow[:, bt0 + half * MMN:bt0 + (half + 1) * MMN],
                    start=False,
                    stop=True,
                )
            hq = psum.tile([128, TBLK], f32, tag="hq")
            for half in range(2):
                psl = slice(half * MMN, (half + 1) * MMN)
                for dc in range(DC):
                    csl = slice(dc * TBLK + half * MMN, dc * TBLK + (half + 1) * MMN)
                    nc.tensor.matmul(
                        hq[:, psl],
                        lhsT=wh_hc[hc][:, dc * 128:(dc + 1) * 128],
                        rhs=xh[:, csl],
                        start=(dc == 0),
                        stop=(dc == DC - 1),
                    )
            state[i] = {"zq": zq, "hq": hq}

        def emit_zs(i):
            """ACT: z = sigmoid(zq) (evacuates z psum), a = 1 - z."""
            b, tb, hc, nsub = sched[i]
            st = state[i]
            z = work.tile([128, TBLK], bf16, tag="z")
            a = ab_pool.tile([128, TBLK], bf16, tag="a")
            width = TBLK // nsub
            for sub in range(nsub):
                ssl = slice(sub * width, (sub + 1) * width)
                nc.scalar.activation(z[:, ssl], st["zq"][:, ssl], AF.Sigmoid)
                nc.scalar.activation(
                    a[:, ssl], z[:, ssl], AF.Identity, bias=1.0, scale=-1.0
                )
            st["z"], st["a"] = z, a

        def emit_back(i):
            """DVE: b = (hq + bh) * z (evacuates h psum), scan, out-DMA."""
            b, tb, hc, nsub = sched[i]
            st = state[i]
            z, a = st["z"], st["a"]
            bb = ab_pool.tile([128, TBLK], bf16, tag="b")
            h = h_pool.tile([128, TBLK], bf16, tag="h")
            width = TBLK // nsub
            for sub in range(nsub):
                ssl = slice(sub * width, (sub + 1) * width)
                nc.vector.scalar_tensor_tensor(
                    bb[:, ssl], st["hq"][:, ssl], bh_col[hc], z[:, ssl],
                    op0=OP.add, op1=OP.mult,
                )
                init = (
                    (0.0 if tb == 0 else h_prev[b][hc][:, TBLK - 1:TBLK])
                    if sub == 0 else h[:, sub * width - 1:sub * width]
                )
                nc.vector.tensor_tensor_scan(
                    h[:, ssl], a[:, ssl], bb[:, ssl], init,
                    op0=OP.mult, op1=OP.add,
                )
                if nsub > 1:
                    osl = slice(tb * TBLK + sub * width,
                                tb * TBLK + (sub + 1) * width)
                    nc.sync.dma_start(out=out_ext[b, hc, :, osl], in_=h[:, ssl])
            h_prev[b][hc] = h
            if nsub == 1:
                ts = slice(tb * TBLK, (tb + 1) * TBLK)
                nc.sync.dma_start(out=out_ext[b, hc, :, ts], in_=h[:])
            state[i] = None

        # Tensor runs one tile ahead; ACT (sigma/a) and DVE (b/scan) trail.
        emit_mms(0)
        for i in range(NT):
            if i + 1 < NT:
                emit_mms(i + 1)
            emit_zs(i)
            emit_back(i)

    nc.compile()
    return nc


def _prep_inputs(x, motion_mag, Wz, bz, Wh, bh, motion_weight, motion_bias, alpha):
    import ml_dtypes

    bf = ml_dtypes.bfloat16
    x = np.ascontiguousarray(np.asarray(x, dtype=np.float32))
    mm = np.asarray(motion_mag, dtype=np.float32)
    Wz = np.asarray(Wz, dtype=np.float32)
    Wh = np.asarray(Wh, dtype=np.float32)
    bz = np.asarray(bz, dtype=np.float32)
    bh = np.asarray(bh, dtype=np.float32).reshape(HC, 128, 1)
    mw = float(np.asarray(motion_weight))
    mb = float(np.asarray(motion_bias))
    al = float(np.asarray(alpha))

    a_sp = float(np.log1p(np.exp(al)))  # softplus(alpha)
    sig = 1.0 / (1.0 + np.exp(-(mw * mm + mb)))
    invtau = (1.0 / (1.0 + a_sp * sig)).astype(np.float32)  # [B, T]

    wzt = np.ascontiguousarray(
        Wz.T.reshape(DC, 128, HC, 128).transpose(2, 1, 0, 3)).astype(bf)
    wht = np.ascontiguousarray(
        Wh.T.reshape(DC, 128, HC, 128).transpose(2, 1, 0, 3)).astype(bf)
    bzr = np.ascontiguousarray(bz.reshape(HC, 1, 128)).astype(bf)

    in_maps = []
    for c in range(NCORES):
        xl = x[c * BL:(c + 1) * BL].reshape(BL * T, D)
        xt = np.ascontiguousarray(xl.T)                      # [D, BT] f32
        itc = np.ascontiguousarray(
            invtau[c * BL:(c + 1) * BL]).reshape(1, BT)      # [1, BT]
        # [D, BT] -> [nblk, DC, 128, TBLK]: block-contiguous for 1-DMA loads
        xzt = np.ascontiguousarray(
            (xt * itc).astype(bf).reshape(DC, 128, BL * NTB, TBLK)
            .transpose(2, 0, 1, 3))
        xht = np.ascontiguousarray(
            xt.astype(bf).reshape(DC, 128, BL * NTB, TBLK).transpose(2, 0, 1, 3))
        in_maps.append({
            "xz": xzt,
            "xh": xht,
            "wzt": wzt,
            "wht": wht,
            "bzr": bzr,
            "bh": bh,
            "invtau": itc.astype(bf),
        })
    return in_maps


def _assemble(results):
    outs = []
    for c in range(NCORES):
        o = results[c]["out"]  # [BL, HC, 128, T] bf16
        o = np.transpose(o.astype(np.float32), (0, 3, 1, 2)).reshape(BL, T, H)
        outs.append(o)
    return np.ascontiguousarray(np.concatenate(outs, axis=0))


def _run(inputs, trace=False):
    from concourse.bass_utils import run_bass_kernel_spmd

    bha = np.asarray(inputs["bh"], dtype=np.float32).reshape(-1)
    bh0 = float(bha[0]) if np.all(bha == bha[0]) else None
    key = ("nc", bh0)
    if key not in _CACHE:
        _CACHE[key] = _build_nc(bh0)
    nc = _CACHE[key]
    in_maps = _prep_inputs(**inputs)
    res = run_bass_kernel_spmd(nc, in_maps, list(range(NCORES)), trace=trace)
    return _assemble(res.results), res


def kernel(**inputs):
    out, _ = _run(inputs, trace=False)
    return out


# revision 17
# speedup vs baseline: 1.0720x; 1.0166x over previous
"""MinGRU cell kernel for Trainium2 (8 NeuronCores, data-parallel over batch).

Computes, for x:[B,T,D], motion_mag:[B,T]:
    tau = 1 + softplus(alpha) * sigmoid(mw*mm + mb)        (per b,t)
    z   = sigmoid((x @ Wz^T + bz) / tau)                   (B,T,H)
    ht  = x @ Wh^T + bh                                    (B,T,H)
    h_t = (1-z_t)*h_{t-1} + z_t*ht_t   (scan over t, h_0=0)

Strategy:
  - Shard B=32 across 8 cores (4 per core). Weights replicated (bf16).
  - On-chip layout: h on partitions, t on the free dim, so the recurrence is
    a HW tensor_tensor_scan per [128h, 1024t] tile, carried across t-tiles via
    initial=prev[:, -1:].
  - The gate temperature is folded into the z-GEMM so no device op ever
    touches invtau: the host uploads x_z = x*invtau (bf16) for the z
    projection and x_h = x (bf16) for the candidate projection, and the
    column-dependent bias bz*invtau rides as a 513th contraction row
    (K=1 matmul: lhsT = bz row, rhs = invtau row). PSUM_z then directly
    holds sigmoid's argument.
  - Post-GEMM dataflow is one-directional with minimal cross-engine
    coupling (the DVE scan is fixed at 2 cyc/elem, so the DVE gets only
    work that must be there):
      ACT     : z = sigmoid(zq)    (PSUM->SBUF bf16; evacuates z psum)
      ACT     : a = 1 - z          (Identity, scale=-1, bias=1)
      DVE STT : b = (hq + bh) * z  (PSUM read; evacuates h psum)
      DVE scan: h = scan(a, b)     (bf16 io, fp32 state)
    Tensor (18 matmuls/tile) is the pacing engine; DVE/ACT run with slack.
  - The output DMA is bf16; the host casts back to fp32.
  - A few dummy fp32 matmuls at t=0 warm the PE HAM clock-gate (2.4 GHz)
    while the first weight/x DMAs land.
  - Host pre-transposes x to [d, b*t] per core and un-transposes the output.
"""

import sys

import numpy as np

if "/opt/trn_rl_repo" not in sys.path:
    sys.path.insert(0, "/opt/trn_rl_repo")

B, T, D, H = 32, 2048, 512, 512
NCORES = 8
BL = B // NCORES            # batch per core = 4
TBLK = 1024                 # t-columns per tile
MMN = 512                   # matmul free-dim (1 psum bank)
NTB = T // TBLK             # 2 t-blocks per sample
DC = D // 128               # 4 contraction chunks
HC = H // 128               # 4 h partition chunks
BT = BL * T                 # 8192 columns per core

_CACHE = {}


def _build_nc(bh0=None):
    import concourse.bass as bass
    import concourse.bacc as bacc
    import concourse.mybir as mybir
    import concourse.tile as tile
    from contextlib import ExitStack

    f32 = mybir.dt.float32
    bf16 = mybir.dt.bfloat16
    AF = mybir.ActivationFunctionType
    OP = mybir.AluOpType

    nc = bacc.Bacc("TRN2", target_bir_lowering=False, debug=False)

    xz_ext = nc.declare_dram_parameter("xz", [BL * NTB, DC, 128, TBLK], bf16, isOutput=False)
    xh_ext = nc.declare_dram_parameter("xh", [BL * NTB, DC, 128, TBLK], bf16, isOutput=False)
    wzt_ext = nc.declare_dram_parameter("wzt", [HC, 128, DC, 128], bf16, isOutput=False)
    wht_ext = nc.declare_dram_parameter("wht", [HC, 128, DC, 128], bf16, isOutput=False)
    bzr_ext = nc.declare_dram_parameter("bzr", [HC, 1, 128], bf16, isOutput=False)
    bh_ext = nc.declare_dram_parameter("bh", [HC, 128, 1], f32, isOutput=False)
    itr_ext = nc.declare_dram_parameter("invtau", [1, BT], bf16, isOutput=False)
    out_ext = nc.declare_dram_parameter("out", [BL, HC, 128, T], bf16, isOutput=True)

    with tile.TileContext(nc) as tc, ExitStack() as ctx:
        singles = ctx.enter_context(tc.tile_pool(name="singles", bufs=1))
        x_pool = ctx.enter_context(tc.tile_pool(name="x", bufs=3))
        psum = ctx.enter_context(tc.tile_pool(name="psum", bufs=2, space="PSUM"))
        work = ctx.enter_context(tc.tile_pool(name="work", bufs=4))
        ab_pool = ctx.enter_context(tc.tile_pool(name="ab", bufs=4))
        h_pool = ctx.enter_context(tc.tile_pool(name="h", bufs=8))

        # HAM warm-up: dependency-free fp32 matmuls (1 col / 4 cycles, so
        # each is long) keep the PE busy while the first weight/x DMAs land,
        # flipping the clock-gate to 8/8 before the real GEMMs start.
        warm = singles.tile([128, MMN], f32, tag="warm", name="warm")
        nc.gpsimd.memset(warm[:], 0.0)
        # Dummy activation: triggers the ~2.7us ACT table load during the
        # initial DMA window instead of on the first tile's critical path.
        warmact = singles.tile([128, 1], bf16, tag="warmact", name="warmact")
        nc.scalar.activation(warmact[:], warm[:, 0:1], AF.Sigmoid)
        wq0 = psum.tile([128, MMN], f32, tag="zq", name="warmq")
        for i in range(4):
            nc.tensor.matmul(
                wq0[:], lhsT=warm[:, 0:128], rhs=warm[:], start=True, stop=True
            )

        # Weights are hc-major in DRAM: the first matmul group (hc=0) only
        # needs a 128KB DMA. First block's x arrives as 512-col halves so the
        # first 4-matmul group is gated on ~0.6MB instead of 2MB.
        wz_hc, wh_hc = [None] * HC, [None] * HC
        def x_half_dma(tile_, ext, k, half):
            # cols [half*MMN, (half+1)*MMN) of every dc chunk of block k, in
            # one strided DMA: dst tile[:, dc*TBLK + half*MMN + c].
            in_ = ext[k, :, :, half * MMN:(half + 1) * MMN]
            in_p = bass.AP(
                tensor=in_.tensor, offset=in_.offset,
                ap=[list(in_.ap[1]), list(in_.ap[0]), list(in_.ap[2])],
            )
            t_ap = tile_[:, half * MMN::TBLK]   # [128, DC] strided view
            out_p = bass.AP(
                tensor=t_ap.tensor, offset=t_ap.offset,
                ap=list(t_ap.ap) + [[1, MMN]],
            )
            nc.sync.dma_start(out=out_p, in_=in_p)

        wz_hc[0] = singles.tile([128, DC * 128], bf16, tag="wzhc0", name="wzhc0")
        nc.sync.dma_start(out=wz_hc[0][:], in_=wzt_ext[0])
        xz0 = x_pool.tile([128, DC * TBLK], bf16, tag="xz", name="xz0")
        x_half_dma(xz0, xz_ext, 0, 0)
        wh_hc[0] = singles.tile([128, DC * 128], bf16, tag="whhc0", name="whhc0")
        nc.sync.dma_start(out=wh_hc[0][:], in_=wht_ext[0])
        xh0 = x_pool.tile([128, DC * TBLK], bf16, tag="xh", name="xh0")
        x_half_dma(xh0, xh_ext, 0, 0)
        # invtau row (partition 0 only) + per-hc bz rows for the bias matmul.
        itrow = singles.tile([1, BT], bf16, tag="itrow", name="itrow")
        nc.sync.dma_start(out=itrow[:, 0:TBLK], in_=itr_ext[:, 0:TBLK])
        bzr = []
        for hc in range(HC):
            r = singles.tile([1, 128], bf16, tag=f"bzr{hc}", name=f"bzr{hc}")
            nc.sync.dma_start(out=r[:], in_=bzr_ext[hc])
            bzr.append(r)
        x_half_dma(xz0, xz_ext, 0, 1)
        x_half_dma(xh0, xh_ext, 0, 1)
        for hc in range(1, HC):
            w = singles.tile([128, DC * 128], bf16, tag=f"wzhc{hc}", name=f"wzhc{hc}")
            nc.sync.dma_start(out=w[:], in_=wzt_ext[hc])
            wz_hc[hc] = w
            w = singles.tile([128, DC * 128], bf16, tag=f"whhc{hc}", name=f"whhc{hc}")
            nc.sync.dma_start(out=w[:], in_=wht_ext[hc])
            wh_hc[hc] = w
        # rest of the invtau row (tiny; after the hot first-block DMAs)
        nc.sync.dma_start(out=itrow[:, TBLK:BT], in_=itr_ext[:, TBLK:BT])

        # b-STT bias: uniform bh rides as an immediate, else [128,1] columns.
        if bh0 is not None:
            bh_col = [bh0] * HC
        else:
            bh_col = []
            for hc in range(HC):
                bc = singles.tile([128, 1], f32, tag=f"bh{hc}", name=f"bh{hc}")
                nc.gpsimd.dma_start(out=bc[:], in_=bh_ext[hc])
                bh_col.append(bc[:])

        # Tile schedule: 32 tiles of [128h, 1024t], hc-inner.
        sched = []
        for b in range(BL):
            for tb in range(NTB):
                for hc in range(HC):
                    first_blk = (b == 0 and tb == 0)
                    last = (b == BL - 1 and tb == NTB - 1 and hc == HC - 1)
                    nsub = (4 if last
                            else 2 if (first_blk and hc == 0) else 1)
                    sched.append((b, tb, hc, nsub))
        NT = len(sched)

        xs_blocks = {}
        state = [None] * NT
        h_prev = [[None] * HC for _ in range(BL)]

        xs_blocks[0] = (xz0, xh0)

        def emit_mms(i):
            """x prefetch (one block ahead) + z-GEMM (with bias row) + h-GEMM."""
            b, tb, hc, nsub = sched[i]
            k = b * NTB + tb
            bt0 = b * T + tb * TBLK
            if hc == 0 and k + 1 < BL * NTB:
                kn = k + 1

                def x_blk_dma(ext):
                    # [DC,128,TBLK] dc-major in DRAM -> [128, DC*TBLK] tile,
                    # partition dim first on both sides of the transfer.
                    xt = x_pool.tile([128, DC * TBLK], bf16,
                                     tag="xz" if ext is xz_ext else "xh")
                    in_ = ext[kn]
                    in_p = bass.AP(
                        tensor=in_.tensor, offset=in_.offset,
                        ap=[list(in_.ap[1]), list(in_.ap[0]), list(in_.ap[2])],
                    )
                    t_ap = xt[:]
                    out_p = bass.AP(
                        tensor=t_ap.tensor, offset=t_ap.offset,
                        ap=[list(t_ap.ap[0]), [TBLK, DC], [1, TBLK]],
                    )
                    nc.sync.dma_start(out=out_p, in_=in_p)
                    return xt

                xs_blocks[kn] = (x_blk_dma(xz_ext), x_blk_dma(xh_ext))
            xz, xh = xs_blocks[k]
            zq = psum.tile([128, TBLK], f32, tag="zq")
            for half in range(2):
                psl = slice(half * MMN, (half + 1) * MMN)
                for dc in range(DC):
                    csl = slice(dc * TBLK + half * MMN, dc * TBLK + (half + 1) * MMN)
                    nc.tensor.matmul(
                        zq[:, psl],
                        lhsT=wz_hc[hc][:, dc * 128:(dc + 1) * 128],
                        rhs=xz[:, csl],
                        start=(dc == 0),
                        stop=False,
                    )
                # bias row: zq += bz ⊗ invtau  (K=1 matmul)
                nc.tensor.matmul(
                    zq[:, psl],
                    lhsT=bzr[hc][:],
                    rhs=itrow[:, bt0 + half * MMN:bt0 + (half + 1) * MMN],
                    start=False,
                    stop=True,
                )
            hq = psum.tile([128, TBLK], f32, tag="hq")
            for half in range(2):
                psl = slice(half * MMN, (half + 1) * MMN)
                for dc in range(DC):
                    csl = slice(dc * TBLK + half * MMN, dc * TBLK + (half + 1) * MMN)
                    nc.tensor.matmul(
                        hq[:, psl],
                        lhsT=wh_hc[hc][:, dc * 128:(dc + 1) * 128],
                        rhs=xh[:, csl],
                        start=(dc == 0),
                        stop=(dc == DC - 1),
                    )
            state[i] = {"zq": zq, "hq": hq}

        def emit_zs(i):
            """ACT: z = sigmoid(zq) (evacuates z psum), a = 1 - z."""
            b, tb, hc, nsub = sched[i]
            st = state[i]
            z = work.tile([128, TBLK], bf16, tag="z")
            a = ab_pool.tile([128, TBLK], bf16, tag="a")
            width = TBLK // nsub
            for sub in range(nsub):
                ssl = slice(sub * width, (sub + 1) * width)
                nc.scalar.activation(z[:, ssl], st["zq"][:, ssl], AF.Sigmoid)
                nc.scalar.activation(
                    a[:, ssl], z[:, ssl], AF.Identity, bias=1.0, scale=-1.0
                )
            st["z"], st["a"] = z, a

        def emit_back(i):
            """DVE: b = (hq + bh) * z (evacuates h psum), scan, out-DMA."""
            b, tb, hc, nsub = sched[i]
            st = state[i]
            z, a = st["z"], st["a"]
            bb = ab_pool.tile([128, TBLK], bf16, tag="b")
            h = h_pool.tile([128, TBLK], bf16, tag="h")
            width = TBLK // nsub
            for sub in range(nsub):
                ssl = slice(sub * width, (sub + 1) * width)
                nc.vector.scalar_tensor_tensor(
                    bb[:, ssl], st["hq"][:, ssl], bh_col[hc], z[:, ssl],
                    op0=OP.add, op1=OP.mult,
                )
                init = (
                    (0.0 if tb == 0 else h_prev[b][hc][:, TBLK - 1:TBLK])
                    if sub == 0 else h[:, sub * width - 1:sub * width]
                )
                nc.vector.tensor_tensor_scan(
                    h[:, ssl], a[:, ssl], bb[:, ssl], init,
                    op0=OP.mult, op1=OP.add,
                )
                if nsub > 1:
                    osl = slice(tb * TBLK + sub * width,
                                tb * TBLK + (sub + 1) * width)
                    nc.sync.dma_start(out=out_ext[b, hc, :, osl], in_=h[:, ssl])
            h_prev[b][hc] = h
            if nsub == 1:
                ts = slice(tb * TBLK, (tb + 1) * TBLK)
                nc.sync.dma_start(out=out_ext[b, hc, :, ts], in_=h[:])
            state[i] = None

        # Tensor runs one tile ahead; ACT (sigma/a) and DVE (b/scan) trail.
        emit_mms(0)
        for i in range(NT):
            if i + 1 < NT:
                emit_mms(i + 1)
            emit_zs(i)
            emit_back(i)

    nc.compile()
    return nc


def _prep_inputs(x, motion_mag, Wz, bz, Wh, bh, motion_weight, motion_bias, alpha):
    import ml_dtypes

    bf = ml_dtypes.bfloat16
    x = np.ascontiguousarray(np.asarray(x, dtype=np.float32))
    mm = np.asarray(motion_mag, dtype=np.float32)
    Wz = np.asarray(Wz, dtype=np.float32)
    Wh = np.asarray(Wh, dtype=np.float32)
    bz = np.asarray(bz, dtype=np.float32)
    bh = np.asarray(bh, dtype=np.float32).reshape(HC, 128, 1)
    mw = float(np.asarray(motion_weight))
    mb = float(np.asarray(motion_bias))
    al = float(np.asarray(alpha))

    a_sp = float(np.log1p(np.exp(al)))  # softplus(alpha)
    sig = 1.0 / (1.0 + np.exp(-(mw * mm + mb)))
    invtau = (1.0 / (1.0 + a_sp * sig)).astype(np.float32)  # [B, T]

    wzt = np.ascontiguousarray(
        Wz.T.reshape(DC, 128, HC, 128).transpose(2, 1, 0, 3)).astype(bf)
    wht = np.ascontiguousarray(
        Wh.T.reshape(DC, 128, HC, 128).transpose(2, 1, 0, 3)).astype(bf)
    bzr = np.ascontiguousarray(bz.reshape(HC, 1, 128)).astype(bf)

    in_maps = []
    for c in range(NCORES):
        xl = x[c * BL:(c + 1) * BL].reshape(BL * T, D)
        xt = np.ascontiguousarray(xl.T)                      # [D, BT] f32
        itc = np.ascontiguousarray(
            invtau[c * BL:(c + 1) * BL]).reshape(1, BT)      # [1, BT]
        # [D, BT] -> [nblk, DC, 128, TBLK]: block-contiguous for 1-DMA loads
        xzt = np.ascontiguousarray(
            (xt * itc).astype(bf).reshape(DC, 128, BL * NTB, TBLK)
            .transpose(2, 0, 1, 3))
        xht = np.ascontiguousarray(
            xt.astype(bf).reshape(DC, 128, BL * NTB, TBLK).transpose(2, 0, 1, 3))
        in_maps.append({
            "xz": xzt,
            "xh": xht,
            "wzt": wzt,
            "wht": wht,
            "bzr": bzr,
            "bh": bh,
            "invtau": itc.astype(bf),
        })
    return in_maps


def _assemble(results):
    outs = []
    for c in range(NCORES):
        o = results[c]["out"]  # [BL, HC, 128, T] bf16
        o = np.transpose(o.astype(np.float32), (0, 3, 1, 2)).reshape(BL, T, H)
        outs.append(o)
    return np.ascontiguousarray(np.concatenate(outs, axis=0))


def _run(inputs, trace=False):
    from concourse.bass_utils import run_bass_kernel_spmd

    bha = np.asarray(inputs["bh"], dtype=np.float32).reshape(-1)
    bh0 = float(bha[0]) if np.all(bha == bha[0]) else None
    key = ("nc", bh0)
    if key not in _CACHE:
        _CACHE[key] = _build_nc(bh0)
    nc = _CACHE[key]
    in_maps = _prep_inputs(**inputs)
    res = run_bass_kernel_spmd(nc, in_maps, list(range(NCORES)), trace=trace)
    return _assemble(res.results), res


def kernel(**inputs):
    out, _ = _run(inputs, trace=False)
    return out


# revision 18
# speedup vs baseline: 1.1037x; 1.0296x over previous
"""MinGRU cell kernel for Trainium2 (8 NeuronCores, data-parallel over batch).

Computes, for x:[B,T,D], motion_mag:[B,T]:
    tau = 1 + softplus(alpha) * sigmoid(mw*mm + mb)        (per b,t)
    z   = sigmoid((x @ Wz^T + bz) / tau)                   (B,T,H)
    ht  = x @ Wh^T + bh                                    (B,T,H)
    h_t = (1-z_t)*h_{t-1} + z_t*ht_t   (scan over t, h_0=0)

Strategy:
  - Shard B=32 across 8 cores (4 per core). Weights replicated (bf16).
  - On-chip layout: h on partitions, t on the free dim, so the recurrence is
    a HW tensor_tensor_scan per [128h, 1024t] tile, carried across t-tiles via
    initial=prev[:, -1:].
  - The gate temperature is folded into the z-GEMM so no device op ever
    touches invtau: the host uploads x_z = x*invtau (bf16) for the z
    projection and x_h = x (bf16) for the candidate projection, and the
    column-dependent bias bz*invtau rides as a 513th contraction row
    (K=1 matmul: lhsT = bz row, rhs = invtau row). PSUM_z then directly
    holds sigmoid's argument.
  - Post-GEMM dataflow is one-directional with minimal cross-engine
    coupling (the DVE scan is fixed at 2 cyc/elem, so the DVE gets only
    work that must be there):
      ACT     : z = sigmoid(zq)    (PSUM->SBUF bf16; evacuates z psum)
      ACT     : a = 1 - z          (Identity, scale=-1, bias=1)
      DVE STT : b = (hq + bh) * z  (PSUM read; evacuates h psum)
      DVE scan: h = scan(a, b)     (bf16 io, fp32 state)
    Tensor (18 matmuls/tile) is the pacing engine; DVE/ACT run with slack.
  - The output DMA is bf16; the host casts back to fp32.
  - A few dummy fp32 matmuls at t=0 warm the PE HAM clock-gate (2.4 GHz)
    while the first weight/x DMAs land.
  - Host pre-transposes x to [d, b*t] per core and un-transposes the output.
"""

import sys

import numpy as np

if "/opt/trn_rl_repo" not in sys.path:
    sys.path.insert(0, "/opt/trn_rl_repo")

B, T, D, H = 32, 2048, 512, 512
NCORES = 8
BL = B // NCORES            # batch per core = 4
TBLK = 1024                 # t-columns per tile
MMN = 512                   # matmul free-dim (1 psum bank)
NTB = T // TBLK             # 2 t-blocks per sample
DC = D // 128               # 4 contraction chunks
HC = H // 128               # 4 h partition chunks
BT = BL * T                 # 8192 columns per core

_CACHE = {}


def _build_nc(bh0=None):
    import concourse.bass as bass
    import concourse.bacc as bacc
    import concourse.mybir as mybir
    import concourse.tile as tile
    from contextlib import ExitStack

    f32 = mybir.dt.float32
    bf16 = mybir.dt.bfloat16
    AF = mybir.ActivationFunctionType
    OP = mybir.AluOpType

    nc = bacc.Bacc("TRN2", target_bir_lowering=False, debug=False)

    xz_ext = nc.declare_dram_parameter("xz", [BL * NTB, DC, 128, TBLK], bf16, isOutput=False)
    xh_ext = nc.declare_dram_parameter("xh", [BL * NTB, DC, 128, TBLK], bf16, isOutput=False)
    wzt_ext = nc.declare_dram_parameter("wzt", [HC, 128, DC, 128], bf16, isOutput=False)
    wht_ext = nc.declare_dram_parameter("wht", [HC, 128, DC, 128], bf16, isOutput=False)
    bzr_ext = nc.declare_dram_parameter("bzr", [HC, 1, 128], bf16, isOutput=False)
    bh_ext = nc.declare_dram_parameter("bh", [HC, 128, 1], f32, isOutput=False)
    itr_ext = nc.declare_dram_parameter("invtau", [1, BT], bf16, isOutput=False)
    out_ext = nc.declare_dram_parameter("out", [BL, HC, 128, T], bf16, isOutput=True)

    with tile.TileContext(nc) as tc, ExitStack() as ctx:
        singles = ctx.enter_context(tc.tile_pool(name="singles", bufs=1))
        x_pool = ctx.enter_context(tc.tile_pool(name="x", bufs=3))
        psum = ctx.enter_context(tc.tile_pool(name="psum", bufs=2, space="PSUM"))
        work = ctx.enter_context(tc.tile_pool(name="work", bufs=4))
        ab_pool = ctx.enter_context(tc.tile_pool(name="ab", bufs=4))
        h_pool = ctx.enter_context(tc.tile_pool(name="h", bufs=8))

        # HAM warm-up: dependency-free fp32 matmuls (1 col / 4 cycles, so
        # each is long) keep the PE busy while the first weight/x DMAs land,
        # flipping the clock-gate to 8/8 before the real GEMMs start.
        warm = singles.tile([128, MMN], f32, tag="warm", name="warm")
        nc.gpsimd.memset(warm[:], 0.0)
        # Dummy activation: triggers the ~2.7us ACT table load during the
        # initial DMA window instead of on the first tile's critical path.
        warmact = singles.tile([128, 1], bf16, tag="warmact", name="warmact")
        nc.scalar.activation(warmact[:], warm[:, 0:1], AF.Sigmoid)
        wq0 = psum.tile([128, MMN], f32, tag="zq", name="warmq")
        for i in range(4):
            nc.tensor.matmul(
                wq0[:], lhsT=warm[:, 0:128], rhs=warm[:], start=True, stop=True
            )

        # Weights are hc-major in DRAM: the first matmul group (hc=0) only
        # needs a 128KB DMA. First block's x arrives as 512-col halves so the
        # first 4-matmul group is gated on ~0.6MB instead of 2MB.
        wz_hc, wh_hc = [None] * HC, [None] * HC
        def x_half_dma(tile_, ext, k, half):
            # cols [half*MMN, (half+1)*MMN) of every dc chunk of block k, in
            # one strided DMA: dst tile[:, dc*TBLK + half*MMN + c].
            in_ = ext[k, :, :, half * MMN:(half + 1) * MMN]
            in_p = bass.AP(
                tensor=in_.tensor, offset=in_.offset,
                ap=[list(in_.ap[1]), list(in_.ap[0]), list(in_.ap[2])],
            )
            t_ap = tile_[:, half * MMN::TBLK]   # [128, DC] strided view
            out_p = bass.AP(
                tensor=t_ap.tensor, offset=t_ap.offset,
                ap=list(t_ap.ap) + [[1, MMN]],
            )
            nc.sync.dma_start(out=out_p, in_=in_p)

        wz_hc[0] = singles.tile([128, DC * 128], bf16, tag="wzhc0", name="wzhc0")
        nc.sync.dma_start(out=wz_hc[0][:], in_=wzt_ext[0])
        xz0 = x_pool.tile([128, DC * TBLK], bf16, tag="xz", name="xz0")
        x_half_dma(xz0, xz_ext, 0, 0)
        wh_hc[0] = singles.tile([128, DC * 128], bf16, tag="whhc0", name="whhc0")
        nc.sync.dma_start(out=wh_hc[0][:], in_=wht_ext[0])
        xh0 = x_pool.tile([128, DC * TBLK], bf16, tag="xh", name="xh0")
        x_half_dma(xh0, xh_ext, 0, 0)
        # invtau row (partition 0 only) + per-hc bz rows for the bias matmul.
        itrow = singles.tile([1, BT], bf16, tag="itrow", name="itrow")
        nc.sync.dma_start(out=itrow[:, 0:TBLK], in_=itr_ext[:, 0:TBLK])
        bzr = []
        for hc in range(HC):
            r = singles.tile([1, 128], bf16, tag=f"bzr{hc}", name=f"bzr{hc}")
            nc.sync.dma_start(out=r[:], in_=bzr_ext[hc])
            bzr.append(r)
        x_half_dma(xz0, xz_ext, 0, 1)
        x_half_dma(xh0, xh_ext, 0, 1)
        for hc in range(1, HC):
            w = singles.tile([128, DC * 128], bf16, tag=f"wzhc{hc}", name=f"wzhc{hc}")
            nc.sync.dma_start(out=w[:], in_=wzt_ext[hc])
            wz_hc[hc] = w
            w = singles.tile([128, DC * 128], bf16, tag=f"whhc{hc}", name=f"whhc{hc}")
            nc.sync.dma_start(out=w[:], in_=wht_ext[hc])
            wh_hc[hc] = w
        # rest of the invtau row (tiny; after the hot first-block DMAs)
        nc.sync.dma_start(out=itrow[:, TBLK:BT], in_=itr_ext[:, TBLK:BT])

        # b-STT bias: uniform bh rides as an immediate, else [128,1] columns.
        if bh0 is not None:
            bh_col = [bh0] * HC
        else:
            bh_col = []
            for hc in range(HC):
                bc = singles.tile([128, 1], f32, tag=f"bh{hc}", name=f"bh{hc}")
                nc.gpsimd.dma_start(out=bc[:], in_=bh_ext[hc])
                bh_col.append(bc[:])

        # Tile schedule: 32 tiles of [128h, 1024t], hc-inner.
        sched = []
        for b in range(BL):
            for tb in range(NTB):
                for hc in range(HC):
                    first_blk = (b == 0 and tb == 0)
                    last = (b == BL - 1 and tb == NTB - 1 and hc == HC - 1)
                    nsub = (4 if last
                            else 2 if (first_blk and hc == 0) else 1)
                    sched.append((b, tb, hc, nsub))
        NT = len(sched)

        xs_blocks = {}
        state = [None] * NT
        h_prev = [[None] * HC for _ in range(BL)]

        xs_blocks[0] = (xz0, xh0)

        def emit_mms(i):
            """x prefetch (one block ahead) + z-GEMM (with bias row) + h-GEMM."""
            b, tb, hc, nsub = sched[i]
            k = b * NTB + tb
            bt0 = b * T + tb * TBLK
            if hc == 0 and k + 1 < BL * NTB:
                kn = k + 1

                def x_blk_dma(ext):
                    # [DC,128,TBLK] dc-major in DRAM -> [128, DC*TBLK] tile,
                    # partition dim first on both sides of the transfer.
                    xt = x_pool.tile([128, DC * TBLK], bf16,
                                     tag="xz" if ext is xz_ext else "xh")
                    in_ = ext[kn]
                    in_p = bass.AP(
                        tensor=in_.tensor, offset=in_.offset,
                        ap=[list(in_.ap[1]), list(in_.ap[0]), list(in_.ap[2])],
                    )
                    t_ap = xt[:]
                    out_p = bass.AP(
                        tensor=t_ap.tensor, offset=t_ap.offset,
                        ap=[list(t_ap.ap[0]), [TBLK, DC], [1, TBLK]],
                    )
                    nc.sync.dma_start(out=out_p, in_=in_p)
                    return xt

                xs_blocks[kn] = (x_blk_dma(xz_ext), x_blk_dma(xh_ext))
            xz, xh = xs_blocks[k]
            zq = psum.tile([128, TBLK], f32, tag="zq")
            for half in range(2):
                psl = slice(half * MMN, (half + 1) * MMN)
                for dc in range(DC):
                    csl = slice(dc * TBLK + half * MMN, dc * TBLK + (half + 1) * MMN)
                    nc.tensor.matmul(
                        zq[:, psl],
                        lhsT=wz_hc[hc][:, dc * 128:(dc + 1) * 128],
                        rhs=xz[:, csl],
                        start=(dc == 0),
                        stop=False,
                    )
            # bias rows last, back to back: one LDWEIGHTS covers both halves
            # and only one weight-row transition stalls the pipeline.
            for half in range(2):
                psl = slice(half * MMN, (half + 1) * MMN)
                nc.tensor.matmul(
                    zq[:, psl],
                    lhsT=bzr[hc][:],
                    rhs=itrow[:, bt0 + half * MMN:bt0 + (half + 1) * MMN],
                    start=False,
                    stop=True,
                )
            hq = psum.tile([128, TBLK], f32, tag="hq")
            for half in range(2):
                psl = slice(half * MMN, (half + 1) * MMN)
                for dc in range(DC):
                    csl = slice(dc * TBLK + half * MMN, dc * TBLK + (half + 1) * MMN)
                    nc.tensor.matmul(
                        hq[:, psl],
                        lhsT=wh_hc[hc][:, dc * 128:(dc + 1) * 128],
                        rhs=xh[:, csl],
                        start=(dc == 0),
                        stop=(dc == DC - 1),
                    )
            state[i] = {"zq": zq, "hq": hq}

        def emit_zs(i):
            """ACT: z = sigmoid(zq) (evacuates z psum), a = 1 - z."""
            b, tb, hc, nsub = sched[i]
            st = state[i]
            z = work.tile([128, TBLK], bf16, tag="z")
            a = ab_pool.tile([128, TBLK], bf16, tag="a")
            width = TBLK // nsub
            for sub in range(nsub):
                ssl = slice(sub * width, (sub + 1) * width)
                nc.scalar.activation(z[:, ssl], st["zq"][:, ssl], AF.Sigmoid)
                nc.scalar.activation(
                    a[:, ssl], z[:, ssl], AF.Identity, bias=1.0, scale=-1.0
                )
            st["z"], st["a"] = z, a

        def emit_back(i):
            """DVE: b = (hq + bh) * z (evacuates h psum), scan, out-DMA."""
            b, tb, hc, nsub = sched[i]
            st = state[i]
            z, a = st["z"], st["a"]
            bb = ab_pool.tile([128, TBLK], bf16, tag="b")
            h = h_pool.tile([128, TBLK], bf16, tag="h")
            width = TBLK // nsub
            for sub in range(nsub):
                ssl = slice(sub * width, (sub + 1) * width)
                nc.vector.scalar_tensor_tensor(
                    bb[:, ssl], st["hq"][:, ssl], bh_col[hc], z[:, ssl],
                    op0=OP.add, op1=OP.mult,
                )
                init = (
                    (0.0 if tb == 0 else h_prev[b][hc][:, TBLK - 1:TBLK])
                    if sub == 0 else h[:, sub * width - 1:sub * width]
                )
                nc.vector.tensor_tensor_scan(
                    h[:, ssl], a[:, ssl], bb[:, ssl], init,
                    op0=OP.mult, op1=OP.add,
                )
                if nsub > 1:
                    osl = slice(tb * TBLK + sub * width,
                                tb * TBLK + (sub + 1) * width)
                    nc.sync.dma_start(out=out_ext[b, hc, :, osl], in_=h[:, ssl])
            h_prev[b][hc] = h
            if nsub == 1:
                ts = slice(tb * TBLK, (tb + 1) * TBLK)
                nc.sync.dma_start(out=out_ext[b, hc, :, ts], in_=h[:])
            state[i] = None

        # Tensor runs one tile ahead; ACT (sigma/a) and DVE (b/scan) trail.
        emit_mms(0)
        for i in range(NT):
            if i + 1 < NT:
                emit_mms(i + 1)
            emit_zs(i)
            emit_back(i)

    nc.compile()
    return nc


def _prep_inputs(x, motion_mag, Wz, bz, Wh, bh, motion_weight, motion_bias, alpha):
    import ml_dtypes

    bf = ml_dtypes.bfloat16
    x = np.ascontiguousarray(np.asarray(x, dtype=np.float32))
    mm = np.asarray(motion_mag, dtype=np.float32)
    Wz = np.asarray(Wz, dtype=np.float32)
    Wh = np.asarray(Wh, dtype=np.float32)
    bz = np.asarray(bz, dtype=np.float32)
    bh = np.asarray(bh, dtype=np.float32).reshape(HC, 128, 1)
    mw = float(np.asarray(motion_weight))
    mb = float(np.asarray(motion_bias))
    al = float(np.asarray(alpha))

    a_sp = float(np.log1p(np.exp(al)))  # softplus(alpha)
    sig = 1.0 / (1.0 + np.exp(-(mw * mm + mb)))
    invtau = (1.0 / (1.0 + a_sp * sig)).astype(np.float32)  # [B, T]

    wzt = np.ascontiguousarray(
        Wz.T.reshape(DC, 128, HC, 128).transpose(2, 1, 0, 3)).astype(bf)
    wht = np.ascontiguousarray(
        Wh.T.reshape(DC, 128, HC, 128).transpose(2, 1, 0, 3)).astype(bf)
    bzr = np.ascontiguousarray(bz.reshape(HC, 1, 128)).astype(bf)

    in_maps = []
    for c in range(NCORES):
        xl = x[c * BL:(c + 1) * BL].reshape(BL * T, D)
        xt = np.ascontiguousarray(xl.T)                      # [D, BT] f32
        itc = np.ascontiguousarray(
            invtau[c * BL:(c + 1) * BL]).reshape(1, BT)      # [1, BT]
        # [D, BT] -> [nblk, DC, 128, TBLK]: block-contiguous for 1-DMA loads
        xzt = np.ascontiguousarray(
            (xt * itc).astype(bf).reshape(DC, 128, BL * NTB, TBLK)
            .transpose(2, 0, 1, 3))
        xht = np.ascontiguousarray(
            xt.astype(bf).reshape(DC, 128, BL * NTB, TBLK).transpose(2, 0, 1, 3))
        in_maps.append({
            "xz": xzt,
            "xh": xht,
            "wzt": wzt,
            "wht": wht,
            "bzr": bzr,
            "bh": bh,
            "invtau": itc.astype(bf),
        })
    return in_maps


def _assemble(results):
    outs = []
    for c in range(NCORES):
        o = results[c]["out"]  # [BL, HC, 128, T] bf16
        o = np.transpose(o.astype(np.float32), (0, 3, 1, 2)).reshape(BL, T, H)
        outs.append(o)
    return np.ascontiguousarray(np.concatenate(outs, axis=0))


def _run(inputs, trace=False):
    from concourse.bass_utils import run_bass_kernel_spmd

    bha = np.asarray(inputs["bh"], dtype=np.float32).reshape(-1)
    bh0 = float(bha[0]) if np.all(bha == bha[0]) else None
    key = ("nc", bh0)
    if key not in _CACHE:
        _CACHE[key] = _build_nc(bh0)
    nc = _CACHE[key]
    in_maps = _prep_inputs(**inputs)
    res = run_bass_kernel_spmd(nc, in_maps, list(range(NCORES)), trace=trace)
    return _assemble(res.results), res


def kernel(**inputs):
    out, _ = _run(inputs, trace=False)
    return out


# revision 19
# speedup vs baseline: 1.2450x; 1.1280x over previous
"""MinGRU cell kernel for Trainium2 (8 NeuronCores, data-parallel over batch).

Computes, for x:[B,T,D], motion_mag:[B,T]:
    tau = 1 + softplus(alpha) * sigmoid(mw*mm + mb)        (per b,t)
    z   = sigmoid((x @ Wz^T + bz) / tau)                   (B,T,H)
    ht  = x @ Wh^T + bh                                    (B,T,H)
    h_t = (1-z_t)*h_{t-1} + z_t*ht_t   (scan over t, h_0=0)

Strategy:
  - Shard B=32 across 8 cores (4 per core). Weights replicated (bf16).
  - On-chip layout: h on partitions, t on the free dim, so the recurrence is
    a HW tensor_tensor_scan per [128h, 1024t] tile, carried across t-tiles via
    initial=prev[:, -1:].
  - The gate temperature is folded into the z-GEMM so no device op ever
    touches invtau: the host uploads x_z = x*invtau (bf16) for the z
    projection and x_h = x (bf16) for the candidate projection, and the
    column-dependent bias bz*invtau rides as a 513th contraction row
    (K=1 matmul: lhsT = bz row, rhs = invtau row). PSUM_z then directly
    holds sigmoid's argument.
  - Post-GEMM dataflow is one-directional with minimal cross-engine
    coupling (the DVE scan is fixed at 2 cyc/elem, so the DVE gets only
    work that must be there):
      ACT     : z = sigmoid(zq)    (PSUM->SBUF bf16; evacuates z psum)
      ACT     : a = 1 - z          (Identity, scale=-1, bias=1)
      DVE STT : b = (hq + bh) * z  (PSUM read; evacuates h psum)
      DVE scan: h = scan(a, b)     (bf16 io, fp32 state)
    Tensor (18 matmuls/tile) is the pacing engine; DVE/ACT run with slack.
  - The output DMA is bf16; the host casts back to fp32.
  - A few dummy fp32 matmuls at t=0 warm the PE HAM clock-gate (2.4 GHz)
    while the first weight/x DMAs land.
  - Host pre-transposes x to [d, b*t] per core and un-transposes the output.
"""

import sys

import numpy as np

if "/opt/trn_rl_repo" not in sys.path:
    sys.path.insert(0, "/opt/trn_rl_repo")

B, T, D, H = 32, 2048, 512, 512
NCORES = 8
BL = B // NCORES            # batch per core = 4
TBLK = 1024                 # t-columns per tile
MMN = 512                   # matmul free-dim (1 psum bank)
NTB = T // TBLK             # 2 t-blocks per sample
DC = D // 128               # 4 contraction chunks
HC = H // 128               # 4 h partition chunks
BT = BL * T                 # 8192 columns per core

_CACHE = {}


def _build_nc(bh0=None):
    import concourse.bass as bass
    import concourse.bacc as bacc
    import concourse.mybir as mybir
    import concourse.tile as tile
    from contextlib import ExitStack

    f32 = mybir.dt.float32
    bf16 = mybir.dt.bfloat16
    fp8 = mybir.dt.float8e4
    DR = mybir.MatmulPerfMode.DoubleRow
    AF = mybir.ActivationFunctionType
    OP = mybir.AluOpType

    nc = bacc.Bacc("TRN2", target_bir_lowering=False, debug=False)

    xz_ext = nc.declare_dram_parameter("xz", [BL * NTB, DC, 128, TBLK], fp8, isOutput=False)
    xh_ext = nc.declare_dram_parameter("xh", [BL * NTB, DC, 128, TBLK], bf16, isOutput=False)
    wzt_ext = nc.declare_dram_parameter("wzt", [HC, 128, DC, 128], fp8, isOutput=False)
    wht_ext = nc.declare_dram_parameter("wht", [HC, 128, DC, 128], bf16, isOutput=False)
    bzr_ext = nc.declare_dram_parameter("bzr", [HC, 1, 128], bf16, isOutput=False)
    bh_ext = nc.declare_dram_parameter("bh", [HC, 128, 1], f32, isOutput=False)
    itr_ext = nc.declare_dram_parameter("invtau", [1, BT], bf16, isOutput=False)
    out_ext = nc.declare_dram_parameter("out", [BL, HC, 128, T], bf16, isOutput=True)

    with tile.TileContext(nc) as tc, ExitStack() as ctx:
        singles = ctx.enter_context(tc.tile_pool(name="singles", bufs=1))
        x_pool = ctx.enter_context(tc.tile_pool(name="x", bufs=3))
        psum = ctx.enter_context(tc.tile_pool(name="psum", bufs=2, space="PSUM"))
        work = ctx.enter_context(tc.tile_pool(name="work", bufs=4))
        ab_pool = ctx.enter_context(tc.tile_pool(name="ab", bufs=4))
        h_pool = ctx.enter_context(tc.tile_pool(name="h", bufs=8))

        # HAM warm-up: dependency-free fp32 matmuls (1 col / 4 cycles, so
        # each is long) keep the PE busy while the first weight/x DMAs land,
        # flipping the clock-gate to 8/8 before the real GEMMs start.
        warm = singles.tile([128, MMN], f32, tag="warm", name="warm")
        nc.gpsimd.memset(warm[:], 0.0)
        # Dummy activation: triggers the ~2.7us ACT table load during the
        # initial DMA window instead of on the first tile's critical path.
        warmact = singles.tile([128, 1], bf16, tag="warmact", name="warmact")
        nc.scalar.activation(warmact[:], warm[:, 0:1], AF.Sigmoid)
        wq0 = psum.tile([128, MMN], f32, tag="zq", name="warmq")
        for i in range(4):
            nc.tensor.matmul(
                wq0[:], lhsT=warm[:, 0:128], rhs=warm[:], start=True, stop=True
            )

        # Weights are hc-major in DRAM: the first matmul group (hc=0) only
        # needs a 128KB DMA. First block's x arrives as 512-col halves so the
        # first 4-matmul group is gated on ~0.6MB instead of 2MB.
        wz_hc, wh_hc = [None] * HC, [None] * HC
        def x_half_dma(tile_, ext, k, half):
            # cols [half*MMN, (half+1)*MMN) of every dc chunk of block k, in
            # one strided DMA: dst tile[:, dc*TBLK + half*MMN + c].
            in_ = ext[k, :, :, half * MMN:(half + 1) * MMN]
            in_p = bass.AP(
                tensor=in_.tensor, offset=in_.offset,
                ap=[list(in_.ap[1]), list(in_.ap[0]), list(in_.ap[2])],
            )
            t_ap = tile_[:, half * MMN::TBLK]   # [128, DC] strided view
            out_p = bass.AP(
                tensor=t_ap.tensor, offset=t_ap.offset,
                ap=list(t_ap.ap) + [[1, MMN]],
            )
            nc.sync.dma_start(out=out_p, in_=in_p)

        wz_hc[0] = singles.tile([128, DC * 128], fp8, tag="wzhc0", name="wzhc0")
        nc.sync.dma_start(out=wz_hc[0][:], in_=wzt_ext[0])
        xz0 = x_pool.tile([128, DC * TBLK], fp8, tag="xz", name="xz0")
        x_half_dma(xz0, xz_ext, 0, 0)
        wh_hc[0] = singles.tile([128, DC * 128], bf16, tag="whhc0", name="whhc0")
        nc.sync.dma_start(out=wh_hc[0][:], in_=wht_ext[0])
        xh0 = x_pool.tile([128, DC * TBLK], bf16, tag="xh", name="xh0")
        x_half_dma(xh0, xh_ext, 0, 0)
        # invtau row (partition 0 only) + per-hc bz rows for the bias matmul.
        itrow = singles.tile([1, BT], bf16, tag="itrow", name="itrow")
        nc.sync.dma_start(out=itrow[:, 0:TBLK], in_=itr_ext[:, 0:TBLK])
        bzr = []
        for hc in range(HC):
            r = singles.tile([1, 128], bf16, tag=f"bzr{hc}", name=f"bzr{hc}")
            nc.sync.dma_start(out=r[:], in_=bzr_ext[hc])
            bzr.append(r)
        x_half_dma(xz0, xz_ext, 0, 1)
        x_half_dma(xh0, xh_ext, 0, 1)
        for hc in range(1, HC):
            w = singles.tile([128, DC * 128], fp8, tag=f"wzhc{hc}", name=f"wzhc{hc}")
            nc.sync.dma_start(out=w[:], in_=wzt_ext[hc])
            wz_hc[hc] = w
            w = singles.tile([128, DC * 128], bf16, tag=f"whhc{hc}", name=f"whhc{hc}")
            nc.sync.dma_start(out=w[:], in_=wht_ext[hc])
            wh_hc[hc] = w
        # rest of the invtau row (tiny; after the hot first-block DMAs)
        nc.sync.dma_start(out=itrow[:, TBLK:BT], in_=itr_ext[:, TBLK:BT])

        # b-STT bias: uniform bh rides as an immediate, else [128,1] columns.
        if bh0 is not None:
            bh_col = [bh0] * HC
        else:
            bh_col = []
            for hc in range(HC):
                bc = singles.tile([128, 1], f32, tag=f"bh{hc}", name=f"bh{hc}")
                nc.gpsimd.dma_start(out=bc[:], in_=bh_ext[hc])
                bh_col.append(bc[:])

        # Tile schedule: 32 tiles of [128h, 1024t], hc-inner.
        sched = []
        for b in range(BL):
            for tb in range(NTB):
                for hc in range(HC):
                    first_blk = (b == 0 and tb == 0)
                    last = (b == BL - 1 and tb == NTB - 1 and hc == HC - 1)
                    nsub = (4 if last
                            else 2 if (first_blk and hc == 0) else 1)
                    sched.append((b, tb, hc, nsub))
        NT = len(sched)

        xs_blocks = {}
        state = [None] * NT
        h_prev = [[None] * HC for _ in range(BL)]

        xs_blocks[0] = (xz0, xh0)

        def emit_mms(i):
            """x prefetch (one block ahead) + z-GEMM (with bias row) + h-GEMM."""
            b, tb, hc, nsub = sched[i]
            k = b * NTB + tb
            bt0 = b * T + tb * TBLK
            if hc == 0 and k + 1 < BL * NTB:
                kn = k + 1

                def x_blk_dma(ext):
                    # [DC,128,TBLK] dc-major in DRAM -> [128, DC*TBLK] tile,
                    # partition dim first on both sides of the transfer.
                    xt = x_pool.tile([128, DC * TBLK],
                                     fp8 if ext is xz_ext else bf16,
                                     tag="xz" if ext is xz_ext else "xh")
                    in_ = ext[kn]
                    in_p = bass.AP(
                        tensor=in_.tensor, offset=in_.offset,
                        ap=[list(in_.ap[1]), list(in_.ap[0]), list(in_.ap[2])],
                    )
                    t_ap = xt[:]
                    out_p = bass.AP(
                        tensor=t_ap.tensor, offset=t_ap.offset,
                        ap=[list(t_ap.ap[0]), [TBLK, DC], [1, TBLK]],
                    )
                    nc.sync.dma_start(out=out_p, in_=in_p)
                    return xt

                xs_blocks[kn] = (x_blk_dma(xz_ext), x_blk_dma(xh_ext))
            xz, xh = xs_blocks[k]
            zq = psum.tile([128, TBLK], f32, tag="zq")
            for half in range(2):
                psl = slice(half * MMN, (half + 1) * MMN)
                for pair in range(DC // 2):
                    # DoubleRow: one fp8 matmul contracts two 128-row chunks.
                    w2 = wz_hc[hc][:, 256 * pair:256 * pair + 129:128]
                    lhsT3 = bass.AP(
                        tensor=w2.tensor, offset=w2.offset,
                        ap=list(w2.ap) + [[1, 128]],
                    )
                    c0 = 2 * pair * TBLK + half * MMN
                    x2 = xz[:, c0:c0 + TBLK + 1:TBLK]
                    rhs3 = bass.AP(
                        tensor=x2.tensor, offset=x2.offset,
                        ap=list(x2.ap) + [[1, MMN]],
                    )
                    nc.tensor.matmul(
                        zq[:, psl],
                        lhsT=lhsT3,
                        rhs=rhs3,
                        start=(pair == 0),
                        stop=False,
                        perf_mode=DR,
                    )
            # bias rows last, back to back: one LDWEIGHTS covers both halves
            # and only one weight-row transition stalls the pipeline.
            for half in range(2):
                psl = slice(half * MMN, (half + 1) * MMN)
                nc.tensor.matmul(
                    zq[:, psl],
                    lhsT=bzr[hc][:],
                    rhs=itrow[:, bt0 + half * MMN:bt0 + (half + 1) * MMN],
                    start=False,
                    stop=True,
                )
            hq = psum.tile([128, TBLK], f32, tag="hq")
            for half in range(2):
                psl = slice(half * MMN, (half + 1) * MMN)
                for dc in range(DC):
                    csl = slice(dc * TBLK + half * MMN, dc * TBLK + (half + 1) * MMN)
                    nc.tensor.matmul(
                        hq[:, psl],
                        lhsT=wh_hc[hc][:, dc * 128:(dc + 1) * 128],
                        rhs=xh[:, csl],
                        start=(dc == 0),
                        stop=(dc == DC - 1),
                    )
            state[i] = {"zq": zq, "hq": hq}

        def emit_zs(i):
            """ACT: z = sigmoid(zq) (evacuates z psum), a = 1 - z."""
            b, tb, hc, nsub = sched[i]
            st = state[i]
            z = work.tile([128, TBLK], bf16, tag="z")
            a = ab_pool.tile([128, TBLK], bf16, tag="a")
            width = TBLK // nsub
            for sub in range(nsub):
                ssl = slice(sub * width, (sub + 1) * width)
                nc.scalar.activation(
                    z[:, ssl], st["zq"][:, ssl], AF.Sigmoid, scale=1.0 / 4096.0
                )
                nc.scalar.activation(
                    a[:, ssl], z[:, ssl], AF.Identity, bias=1.0, scale=-1.0
                )
            st["z"], st["a"] = z, a

        def emit_back(i):
            """DVE: b = (hq + bh) * z (evacuates h psum), scan, out-DMA."""
            b, tb, hc, nsub = sched[i]
            st = state[i]
            z, a = st["z"], st["a"]
            bb = ab_pool.tile([128, TBLK], bf16, tag="b")
            h = h_pool.tile([128, TBLK], bf16, tag="h")
            width = TBLK // nsub
            for sub in range(nsub):
                ssl = slice(sub * width, (sub + 1) * width)
                nc.vector.scalar_tensor_tensor(
                    bb[:, ssl], st["hq"][:, ssl], bh_col[hc], z[:, ssl],
                    op0=OP.add, op1=OP.mult,
                )
                init = (
                    (0.0 if tb == 0 else h_prev[b][hc][:, TBLK - 1:TBLK])
                    if sub == 0 else h[:, sub * width - 1:sub * width]
                )
                nc.vector.tensor_tensor_scan(
                    h[:, ssl], a[:, ssl], bb[:, ssl], init,
                    op0=OP.mult, op1=OP.add,
                )
                if nsub > 1:
                    osl = slice(tb * TBLK + sub * width,
                                tb * TBLK + (sub + 1) * width)
                    nc.sync.dma_start(out=out_ext[b, hc, :, osl], in_=h[:, ssl])
            h_prev[b][hc] = h
            if nsub == 1:
                ts = slice(tb * TBLK, (tb + 1) * TBLK)
                nc.sync.dma_start(out=out_ext[b, hc, :, ts], in_=h[:])
            state[i] = None

        # Tensor runs one tile ahead; ACT (sigma/a) and DVE (b/scan) trail.
        emit_mms(0)
        for i in range(NT):
            if i + 1 < NT:
                emit_mms(i + 1)
            emit_zs(i)
            emit_back(i)

    nc.compile()
    return nc


def _prep_inputs(x, motion_mag, Wz, bz, Wh, bh, motion_weight, motion_bias, alpha):
    import ml_dtypes

    bf = ml_dtypes.bfloat16
    x = np.ascontiguousarray(np.asarray(x, dtype=np.float32))
    mm = np.asarray(motion_mag, dtype=np.float32)
    Wz = np.asarray(Wz, dtype=np.float32)
    Wh = np.asarray(Wh, dtype=np.float32)
    bz = np.asarray(bz, dtype=np.float32)
    bh = np.asarray(bh, dtype=np.float32).reshape(HC, 128, 1)
    mw = float(np.asarray(motion_weight))
    mb = float(np.asarray(motion_bias))
    al = float(np.asarray(alpha))

    a_sp = float(np.log1p(np.exp(al)))  # softplus(alpha)
    sig = 1.0 / (1.0 + np.exp(-(mw * mm + mb)))
    invtau = (1.0 / (1.0 + a_sp * sig)).astype(np.float32)  # [B, T]

    f8 = ml_dtypes.float8_e4m3
    wzt = np.ascontiguousarray(
        Wz.T.reshape(DC, 128, HC, 128).transpose(2, 1, 0, 3) * 256.0).astype(f8)
    wht = np.ascontiguousarray(
        Wh.T.reshape(DC, 128, HC, 128).transpose(2, 1, 0, 3)).astype(bf)
    bzr = np.ascontiguousarray(bz.reshape(HC, 1, 128) * 4096.0).astype(bf)

    in_maps = []
    for c in range(NCORES):
        xl = x[c * BL:(c + 1) * BL].reshape(BL * T, D)
        xt = np.ascontiguousarray(xl.T)                      # [D, BT] f32
        itc = np.ascontiguousarray(
            invtau[c * BL:(c + 1) * BL]).reshape(1, BT)      # [1, BT]
        # [D, BT] -> [nblk, DC, 128, TBLK]: block-contiguous for 1-DMA loads
        xzt = np.ascontiguousarray(
            (xt * itc * 16.0).astype(f8).reshape(DC, 128, BL * NTB, TBLK)
            .transpose(2, 0, 1, 3))
        xht = np.ascontiguousarray(
            xt.astype(bf).reshape(DC, 128, BL * NTB, TBLK).transpose(2, 0, 1, 3))
        in_maps.append({
            "xz": xzt,
            "xh": xht,
            "wzt": wzt,
            "wht": wht,
            "bzr": bzr,
            "bh": bh,
            "invtau": itc.astype(bf),
        })
    return in_maps


def _assemble(results):
    outs = []
    for c in range(NCORES):
        o = results[c]["out"]  # [BL, HC, 128, T] bf16
        o = np.transpose(o.astype(np.float32), (0, 3, 1, 2)).reshape(BL, T, H)
        outs.append(o)
    return np.ascontiguousarray(np.concatenate(outs, axis=0))


def _run(inputs, trace=False):
    from concourse.bass_utils import run_bass_kernel_spmd

    bha = np.asarray(inputs["bh"], dtype=np.float32).reshape(-1)
    bh0 = float(bha[0]) if np.all(bha == bha[0]) else None
    key = ("nc", bh0)
    if key not in _CACHE:
        _CACHE[key] = _build_nc(bh0)
    nc = _CACHE[key]
    in_maps = _prep_inputs(**inputs)
    res = run_bass_kernel_spmd(nc, in_maps, list(range(NCORES)), trace=trace)
    return _assemble(res.results), res


def kernel(**inputs):
    out, _ = _run(inputs, trace=False)
    return out
